# revision 8
# baseline (speedup 1.0000x reference)
"""Bass/Trainium2 kernel for batched multi-head self-attention.

Module math (per batch b):
    q = vec @ Wq; k = vec @ Wk; v = vec @ Wv            (per head h, dim d=16)
    S = q k^T / sqrt(d);  P = softmax_j(S);  recv = P v
    out = recv @ Wo

Sharding: data-parallel over batch (8 batches -> 8 NeuronCores), weights
replicated. Each core runs an identical Bass program on its vec slice.

Per-head pipeline on a core:
  1. form1: S[i, j] via K=64 zero-padded bf16 matmuls; DVE row-max (negated).
  2. "m-dance": the per-row -max vector is transposed (PE) and DMA-flattened
     into an augmentation partition of the fp32 Q^T tensor.
  3. S'^T[j, i] = KT-aug.T @ QT-aug (K=64 zero-padded fp32): the aug row
     (ones x -max) subtracts the row max inside the matmul, so ACT exp with
     scale=1/4 needs no per-column bias. exp -> P^T in fp16.
  4. PV: lhsT = [V_h | 1] fp16 (M=17) accumulates recv^T plus the softmax
     denominator in one stream, col-tiled across 3 PSUM strips.
  5. Tail: reciprocal + PE expand-matmul + fused normalize, Wo projection.

K=64 zero-padding costs no extra PE streams (same pass count) but keeps the
PE HAM activity monitor warm (2.4 GHz); K=16 matmuls read as ~13% array
activity and leave the PE throttled at 1.2 GHz.

Shapes (hardcoded): vec [8, 1024, 128]; Wq/Wk/Wv [128, 8, 16]; Wo [8, 16, 128].
"""

import sys

sys.path.insert(0, "/opt/trn_rl_repo")

from contextlib import ExitStack

import numpy as np

import concourse.bacc as bacc
import concourse.tile as tile
from concourse import mybir
from concourse.bass_utils import run_bass_kernel_spmd
from concourse.masks import make_identity

F32 = mybir.dt.float32
F16 = mybir.dt.float16
BF16 = mybir.dt.bfloat16
Exp = mybir.ActivationFunctionType.Exp

B, N, X, H, D = 8, 1024, 128, 8, 16
NCHUNK = N // 128          # 8 chunks of 128 along the token dim
SCALE = 0.25               # 1/sqrt(16)
NR = 4                     # qk rounds: 2 heads each at strips {0, 64}

_CACHED_NC = None


def build_nc():
    """Build the per-core Bass program (identical on all cores)."""
    nc = bacc.Bacc("TRN2")

    # DRAM I/O. Weight tensors arrive pre-permuted from numpy (see kernel()).
    d_wq = [nc.dram_tensor(f"wq{r}", (X, 128), F32, kind="ExternalInput")
            for r in range(NR)]
    d_wk = [nc.dram_tensor(f"wk{r}", (X, 128), F32, kind="ExternalInput")
            for r in range(NR)]
    d_wv = nc.dram_tensor("wv", (X, 128), F32, kind="ExternalInput")
    d_wo = nc.dram_tensor("wo", (128, X), F32, kind="ExternalInput")
    d_vec = nc.dram_tensor("vec", (N, X), F32, kind="ExternalInput")
    d_e8 = nc.dram_tensor("e8c", (H, 128), F32, kind="ExternalInput")
    d_ones = nc.dram_tensor("ones", (1, N), F32, kind="ExternalInput")
    d_out = nc.dram_tensor("out", (N, X), F32, kind="ExternalOutput")

    with tile.TileContext(nc) as tc, ExitStack() as top:
        const = top.enter_context(tc.tile_pool(name="const", bufs=1))
        ident = const.tile([128, 128], F32)
        make_identity(nc, ident)

        w_sb = {}
        for name, dram in ([(f"wq{r}", d_wq[r]) for r in range(NR)]
                           + [(f"wk{r}", d_wk[r]) for r in range(NR)]
                           + [("wv", d_wv), ("wo", d_wo)]):
            t = const.tile([128, 128], F32, tag=f"w_{name}", name=f"w_{name}")
            nc.sync.dma_start(out=t[:], in_=dram[:, :])
            w_sb[name] = t

        vecT = const.tile([128, N], F32, tag="vecT")      # [x, n]
        # QT/KT layout per round r (heads 2r, 2r+1): strip t=h%2 occupies
        # partitions [64t, 64t+17): rows 64t+d hold head dim d, row 64t+16
        # is the aug row (ones for KT, -rowmax for QT); rows 64t+17..64t+63
        # are zeros (K=64 padding).
        QT = {r: const.tile([128, N], F32, tag=f"qt{r}", name=f"qt{r}")
              for r in range(NR)}
        KT = {r: const.tile([128, N], F32, tag=f"kt{r}", name=f"kt{r}")
              for r in range(NR)}
        QTh = {r: const.tile([128, N], BF16, tag=f"qth{r}", name=f"qth{r}")
               for r in range(NR)}
        KTh = {r: const.tile([128, N], BF16, tag=f"kth{r}", name=f"kth{r}")
               for r in range(NR)}
        # V layout: [128 j-in-chunk, jc, 17*h + d], col 17h+16 = ones.
        V_sb = const.tile([128, NCHUNK, 17 * H], F16, tag="vsb")
        pt_pool = top.enter_context(tc.tile_pool(name="pt", bufs=2))
        # raw recv output (incl. den rows): head h -> raw[h//3],
        # psum col strip 32*(h%3).
        raw = {r: const.tile([128, N], F32, tag=f"raw{r}", name=f"raw{r}")
               for r in range(3)}
        recvT = const.tile([128, N], F32, tag="recvT")     # [(h d), i]
        recvN = const.tile([128, N], F32, tag="recvN")     # normalized
        den_sb = const.tile([H, N], F32, tag="den")
        rden = const.tile([H, N], F32, tag="rden")
        e8 = const.tile([H, 128], F32, tag="e8")           # expand matrix
        mha_sb = const.tile([128, NCHUNK, X], F32, tag="mha")

        nc.sync.dma_start(out=e8[:], in_=d_e8[:, :])
        v_heads = V_sb[:].rearrange("p c (h s) -> p c h s", h=H)
        nc.vector.memset(v_heads[:, :, :, 16:17], 1.0)

        # ---- Phase 0: vecT via PE transposes; projections. ----
        with tc.tile_pool(name="stage", bufs=3) as stage, \
                tc.tile_pool(name="ps0", bufs=2, space="PSUM") as ps0, \
                tc.tile_pool(name="ps0b", bufs=2, space="PSUM") as ps0b:
            for c in range(NCHUNK):
                vt = stage.tile([128, 128], F32, tag="vstage")
                nc.sync.dma_start(out=vt[:], in_=d_vec[c * 128:(c + 1) * 128, :])
                pt_ = ps0b.tile([128, 128], F32, tag="trp")
                nc.tensor.transpose(pt_[:, :], vt[:], ident[:])
                nc.scalar.copy(vecT[:, c * 128:(c + 1) * 128], pt_[:, :])

            # QT/KT projections: psum = W.T @ vecT  -> [hd-pos, n]
            for rnd in range(NR):
                for wname, dst, dsth in ((f"wq{rnd}", QT[rnd], QTh[rnd]),
                                         (f"wk{rnd}", KT[rnd], KTh[rnd])):
                    p = ps0.tile([128, N], F32, tag="proj")
                    for half in range(2):
                        sl = slice(half * 512, (half + 1) * 512)
                        nc.tensor.matmul(p[:, sl], w_sb[wname][:],
                                         vecT[:, sl], start=True, stop=True)
                    nc.scalar.copy(dst[:, :], p[:, :])
                    nc.vector.tensor_copy(dsth[:, :], p[:, :])
            # ones rows of KT aug partitions
            for rnd in range(NR):
                for t in range(2):
                    nc.sync.dma_start(
                        out=KT[rnd][64 * t + 16:64 * t + 17, :],
                        in_=d_ones[:, :])

            # V projection: per chunk [j, hd] = vecT[:,chunk].T @ Wv
            for c in range(NCHUNK):
                pv = ps0.tile([128, 128], F32, tag="projv")
                nc.tensor.matmul(pv[:, :], vecT[:, c * 128:(c + 1) * 128],
                                 w_sb["wv"][:], start=True, stop=True)
                dst = V_sb[:, c, :].rearrange("p (h s) -> p h s", h=H)
                src = pv[:, :].rearrange("p (h d) -> p h d", h=H)
                nc.vector.tensor_copy(dst[:, :, 0:16], src[:])

        # ---- Main loop over heads. ----
        with tc.tile_pool(name="small", bufs=3) as small, \
                tc.tile_pool(name="psm", bufs=3, space="PSUM") as psm, \
                tc.tile_pool(name="psr", bufs=2, space="PSUM") as psr:
            for h in range(H):
                rnd, t = h // 2, h % 2
                sp = 64 * t
                qt, kt = QT[rnd], KT[rnd]
                qth, kth = QTh[rnd], KTh[rnd]

                # form1: S[i, j], K=64 zero-padded bf16; row-max -> m_h.
                m_h = small.tile([128, NCHUNK], F32, tag="mh")
                for c in range(NCHUNK):
                    f1 = psm.tile([128, N], F32, tag="big")
                    for half in range(2):
                        sl = slice(half * 512, (half + 1) * 512)
                        nc.tensor.matmul(
                            f1[:, sl],
                            qth[sp:sp + 64, c * 128:(c + 1) * 128],
                            kth[sp:sp + 64, sl], start=True, stop=True)
                    nc.vector.tensor_reduce(
                        m_h[:, c:c + 1], f1[:, :], axis=mybir.AxisListType.X,
                        op=mybir.AluOpType.max, negate=True)

                # m-dance: [128, 8] -> transpose -> [8, 128] -> DMA-flatten
                # into the aug row of QT (value = -rowmax).
                trp = psr.tile([128, 512], F32, tag="recv")
                nc.tensor.transpose(trp[0:NCHUNK, 0:128], m_h[:], ident[:])
                m8 = small.tile([NCHUNK, 128], F32, tag="m8")
                nc.vector.tensor_copy(m8[:], trp[0:NCHUNK, 0:128])
                nc.sync.dma_start(out=qt[sp + 16:sp + 17, :], in_=m8[:])

                # S'^T tiles (K=64 zero-padded fp32, aug row) + exp -> PT.
                PT = pt_pool.tile([128, NCHUNK * N], F16, tag="pt")
                for jc in range(NCHUNK):
                    st = psm.tile([128, N], F32, tag="big")
                    for half in range(2):
                        sl = slice(half * 512, (half + 1) * 512)
                        nc.tensor.matmul(
                            st[:, sl],
                            kt[sp:sp + 64, jc * 128:(jc + 1) * 128],
                            qt[sp:sp + 64, sl], start=True, stop=True)
                    nc.scalar.activation(PT[:, jc * N:jc * N + N], st[:, :],
                                         Exp, bias=0.0, scale=SCALE)

                # PV: recvT_aug[17, i] accumulated over j chunks.
                rv = raw[h // 3]
                cs = 32 * (h % 3)
                for half in range(2):
                    prv = psr.tile([128, 512], F32, tag="recv")
                    for jc in range(NCHUNK):
                        nc.tensor.matmul(
                            prv[cs:cs + 17, :],
                            V_sb[:, jc, 17 * h:17 * h + 17],
                            PT[:, jc * N + half * 512: jc * N + (half + 1) * 512],
                            start=(jc == 0), stop=(jc == NCHUNK - 1))
                    nc.vector.tensor_copy(
                        rv[cs:cs + 17, half * 512:(half + 1) * 512],
                        prv[cs:cs + 17, :])

        # ---- Tail: normalize + output projection. ----
        with tc.tile_pool(name="pst", bufs=2, space="PSUM") as pst, \
                tc.tile_pool(name="pstb", bufs=2, space="PSUM") as pstb:
            for h in range(H):
                rv, cs = raw[h // 3], 32 * (h % 3)
                nc.sync.dma_start(out=recvT[16 * h:16 * h + 16, :],
                                  in_=rv[cs:cs + 16, :])
                nc.sync.dma_start(out=den_sb[h:h + 1, :],
                                  in_=rv[cs + 16:cs + 17, :])
            nc.vector.reciprocal(rden[:], den_sb[:])
            pe_ = pst.tile([128, N], F32, tag="expand")
            for half in range(2):
                sl = slice(half * 512, (half + 1) * 512)
                nc.tensor.matmul(pe_[:, sl], e8[:], rden[:, sl],
                                 start=True, stop=True)
            nc.vector.tensor_mul(recvN[:], recvT[:], pe_[:, :])
            for c in range(NCHUNK):
                po = pstb.tile([128, 128], F32, tag="mha")
                nc.tensor.matmul(po[:, :], recvN[:, c * 128:(c + 1) * 128],
                                 w_sb["wo"][:], start=True, stop=True)
                nc.scalar.copy(mha_sb[:, c, :], po[:, :])
                nc.sync.dma_start(out=d_out[c * 128:(c + 1) * 128, :],
                                  in_=mha_sb[:, c, :])

    nc.finalize()
    return nc


def _permute_weights(Wq, Wk, Wv, Wo):
    """Numpy-side weight layout prep: strip-pack with K=64 zero padding."""
    def strip_pack(W, heads):
        out = np.zeros((X, 128), dtype=np.float32)
        for t, h in enumerate(heads):
            out[:, 64 * t:64 * t + 16] = W[:, h, :]
        return out

    e8c = np.zeros((H, 128), dtype=np.float32)
    for h in range(H):
        e8c[h, 16 * h:16 * h + 16] = 1.0
    d = dict(
        wv=np.ascontiguousarray(Wv.reshape(X, 128)),
        wo=np.ascontiguousarray(Wo.reshape(128, X)),
        e8c=e8c, ones=np.ones((1, N), dtype=np.float32),
    )
    for r in range(NR):
        d[f"wq{r}"] = strip_pack(Wq, [2 * r, 2 * r + 1])
        d[f"wk{r}"] = strip_pack(Wk, [2 * r, 2 * r + 1])
    return d


def kernel(Wq, Wk, Wv, Wo, vec, trace=False):
    global _CACHED_NC
    if _CACHED_NC is None:
        _CACHED_NC = build_nc()
    nc = _CACHED_NC

    w = _permute_weights(np.asarray(Wq, np.float32), np.asarray(Wk, np.float32),
                         np.asarray(Wv, np.float32), np.asarray(Wo, np.float32))
    vec = np.asarray(vec, np.float32)
    in_maps = [dict(w, vec=np.ascontiguousarray(vec[b])) for b in range(B)]
    res = run_bass_kernel_spmd(nc, in_maps, core_ids=list(range(B)),
                               trace=trace)
    out = np.stack([res.results[b]["out"] for b in range(B)])
    if trace:
        return out, res
    return out


# revision 9
# speedup vs baseline: 1.7006x; 1.7006x over previous
"""Bass/Trainium2 kernel for batched multi-head self-attention.

Module math (per batch b):
    q = vec @ Wq; k = vec @ Wk; v = vec @ Wv            (per head h, dim d=16)
    S = q k^T / sqrt(d);  P = softmax_j(S);  recv = P v
    out = recv @ Wo

Sharding: data-parallel over batch (8 batches -> 8 NeuronCores), weights
replicated. Each core runs an identical Bass program on its vec slice.

Per-head pipeline on a core:
  1. form1: S[i, j] via K=64 zero-padded bf16 matmuls; DVE row-max (negated).
  2. "m-dance": the per-row -max vector is transposed (PE) and DMA-flattened
     into an augmentation partition of the fp32 Q^T tensor.
  3. S'^T[j, i] = KT-aug.T @ QT-aug (K=64 zero-padded fp32): the aug row
     (ones x -max) subtracts the row max inside the matmul, so ACT exp with
     scale=1/4 needs no per-column bias. exp -> P^T in fp16.
  4. PV: lhsT = [V_h | 1] fp16 (M=17) accumulates recv^T plus the softmax
     denominator in one stream, col-tiled across 3 PSUM strips.
  5. Tail: reciprocal + PE expand-matmul + fused normalize, Wo projection.

K=64 zero-padding costs no extra PE streams (same pass count) but keeps the
PE HAM activity monitor warm (2.4 GHz); K=16 matmuls read as ~13% array
activity and leave the PE throttled at 1.2 GHz.

Shapes (hardcoded): vec [8, 1024, 128]; Wq/Wk/Wv [128, 8, 16]; Wo [8, 16, 128].
"""

import sys

sys.path.insert(0, "/opt/trn_rl_repo")

from contextlib import ExitStack

import numpy as np

import concourse.bacc as bacc
import concourse.tile as tile
from concourse import mybir
from concourse.bass_utils import run_bass_kernel_spmd
from concourse.masks import make_identity

F32 = mybir.dt.float32
F16 = mybir.dt.float16
BF16 = mybir.dt.bfloat16
Exp = mybir.ActivationFunctionType.Exp

B, N, X, H, D = 8, 1024, 128, 8, 16
NCHUNK = N // 128          # 8 chunks of 128 along the token dim
SCALE = 0.25               # 1/sqrt(16)
NR = 4                     # qk rounds: 2 heads each at strips {0, 64}

_CACHED_NC = None


def build_nc():
    """Build the per-core Bass program (identical on all cores)."""
    nc = bacc.Bacc("TRN2")

    # DRAM I/O. Weight tensors arrive pre-permuted from numpy (see kernel()).
    d_wq = [nc.dram_tensor(f"wq{r}", (X, 128), F32, kind="ExternalInput")
            for r in range(NR)]
    d_wk = [nc.dram_tensor(f"wk{r}", (X, 128), F32, kind="ExternalInput")
            for r in range(NR)]
    d_wv = nc.dram_tensor("wv", (X, 128), F32, kind="ExternalInput")
    d_wo = nc.dram_tensor("wo", (128, X), F32, kind="ExternalInput")
    d_vec = nc.dram_tensor("vec", (N, X), F32, kind="ExternalInput")
    d_e8 = nc.dram_tensor("e8c", (H, 128), F32, kind="ExternalInput")
    d_ones = nc.dram_tensor("ones", (1, N), F32, kind="ExternalInput")
    d_out = nc.dram_tensor("out", (N, X), F32, kind="ExternalOutput")

    with tile.TileContext(nc) as tc, ExitStack() as top:
        const = top.enter_context(tc.tile_pool(name="const", bufs=1))
        ident = const.tile([128, 128], F32)
        make_identity(nc, ident)

        w_sb = {}
        for name, dram in ([(f"wq{r}", d_wq[r]) for r in range(NR)]
                           + [(f"wk{r}", d_wk[r]) for r in range(NR)]
                           + [("wv", d_wv), ("wo", d_wo)]):
            t = const.tile([128, 128], F32, tag=f"w_{name}", name=f"w_{name}")
            nc.sync.dma_start(out=t[:], in_=dram[:, :])
            w_sb[name] = t

        vecT = const.tile([128, N], F32, tag="vecT")      # [x, n]
        # QT/KT layout per round r (heads 2r, 2r+1): strip t=h%2 occupies
        # partitions [64t, 64t+17): rows 64t+d hold head dim d, row 64t+16
        # is the aug row (ones for KT, -rowmax for QT); rows 64t+17..64t+63
        # are zeros (K=64 padding).
        QT = {r: const.tile([128, N], F32, tag=f"qt{r}", name=f"qt{r}")
              for r in range(NR)}
        KT = {r: const.tile([128, N], F32, tag=f"kt{r}", name=f"kt{r}")
              for r in range(NR)}
        QTh = {r: const.tile([128, N], BF16, tag=f"qth{r}", name=f"qth{r}")
               for r in range(NR)}
        KTh = {r: const.tile([128, N], BF16, tag=f"kth{r}", name=f"kth{r}")
               for r in range(NR)}
        # V layout: [128 j-in-chunk, jc, 17*h + d], col 17h+16 = ones.
        V_sb = const.tile([128, NCHUNK, 17 * H], F16, tag="vsb")
        pt_pool = top.enter_context(tc.tile_pool(name="pt", bufs=3))
        # raw recv output (incl. den rows): head h -> raw[h//3],
        # psum col strip 32*(h%3).
        raw = {r: const.tile([128, N], F32, tag=f"raw{r}", name=f"raw{r}")
               for r in range(3)}
        recvT = const.tile([128, N], F32, tag="recvT")     # [(h d), i]
        recvN = const.tile([128, N], F32, tag="recvN")     # normalized
        den_sb = const.tile([H, N], F32, tag="den")
        rden = const.tile([H, N], F32, tag="rden")
        e8 = const.tile([H, 128], F32, tag="e8")           # expand matrix
        mha_sb = const.tile([128, NCHUNK, X], F32, tag="mha")

        nc.sync.dma_start(out=e8[:], in_=d_e8[:, :])
        v_heads = V_sb[:].rearrange("p c (h s) -> p c h s", h=H)
        nc.vector.memset(v_heads[:, :, :, 16:17], 1.0)

        # ---- Phase 0: vecT via PE transposes; projections. ----
        with tc.tile_pool(name="stage", bufs=3) as stage, \
                tc.tile_pool(name="ps0", bufs=2, space="PSUM") as ps0, \
                tc.tile_pool(name="ps0b", bufs=2, space="PSUM") as ps0b:
            for c in range(NCHUNK):
                vt = stage.tile([128, 128], F32, tag="vstage")
                nc.sync.dma_start(out=vt[:], in_=d_vec[c * 128:(c + 1) * 128, :])
                pt_ = ps0b.tile([128, 128], F32, tag="trp")
                nc.tensor.transpose(pt_[:, :], vt[:], ident[:])
                nc.scalar.copy(vecT[:, c * 128:(c + 1) * 128], pt_[:, :])

            # QT/KT projections: psum = W.T @ vecT  -> [hd-pos, n]
            for rnd in range(NR):
                for wname, dst, dsth in ((f"wq{rnd}", QT[rnd], QTh[rnd]),
                                         (f"wk{rnd}", KT[rnd], KTh[rnd])):
                    p = ps0.tile([128, N], F32, tag="proj")
                    for half in range(2):
                        sl = slice(half * 512, (half + 1) * 512)
                        nc.tensor.matmul(p[:, sl], w_sb[wname][:],
                                         vecT[:, sl], start=True, stop=True)
                    nc.scalar.copy(dst[:, :], p[:, :])
                    nc.vector.tensor_copy(dsth[:, :], p[:, :])
            # ones rows of KT aug partitions
            for rnd in range(NR):
                for t in range(2):
                    nc.sync.dma_start(
                        out=KT[rnd][64 * t + 16:64 * t + 17, :],
                        in_=d_ones[:, :])

            # V projection: per chunk [j, hd] = vecT[:,chunk].T @ Wv
            for c in range(NCHUNK):
                pv = ps0.tile([128, 128], F32, tag="projv")
                nc.tensor.matmul(pv[:, :], vecT[:, c * 128:(c + 1) * 128],
                                 w_sb["wv"][:], start=True, stop=True)
                dst = V_sb[:, c, :].rearrange("p (h s) -> p h s", h=H)
                src = pv[:, :].rearrange("p (h d) -> p h d", h=H)
                nc.vector.tensor_copy(dst[:, :, 0:16], src[:])

        # ---- Main loop over heads. ----
        with tc.tile_pool(name="small", bufs=3) as small, \
                tc.tile_pool(name="psm", bufs=3, space="PSUM") as psm, \
                tc.tile_pool(name="psr", bufs=2, space="PSUM") as psr:
            for rnd in range(NR):
                pair = (2 * rnd, 2 * rnd + 1)
                qt, kt = QT[rnd], KT[rnd]
                qth, kth = QTh[rnd], KTh[rnd]

                # form1 for both heads, interleaved so LDWEIGHTS of one
                # strip overlaps the other strip's stream.
                m_hs = {}
                for h in pair:
                    m_hs[h] = small.tile([128, NCHUNK], F32, tag="mh",
                                         name=f"mh{h}")
                for c in range(NCHUNK):
                    f1s = {}
                    for h in pair:
                        sp = 64 * (h % 2)
                        f1 = psm.tile([128, N], F32, tag="big",
                                      name=f"f1_{h}_{c}")
                        f1s[h] = f1
                        for half in range(2):
                            sl = slice(half * 512, (half + 1) * 512)
                            nc.tensor.matmul(
                                f1[:, sl],
                                qth[sp:sp + 64, c * 128:(c + 1) * 128],
                                kth[sp:sp + 64, sl], start=True, stop=True)
                    for h in pair:
                        nc.vector.tensor_reduce(
                            m_hs[h][:, c:c + 1], f1s[h][:, :],
                            axis=mybir.AxisListType.X,
                            op=mybir.AluOpType.max, negate=True)

                # m-dance per head: -rowmax -> aug row of QT.
                for h in pair:
                    sp = 64 * (h % 2)
                    trp = psr.tile([128, 512], F32, tag="recv",
                                   name=f"trp{h}")
                    nc.tensor.transpose(trp[0:NCHUNK, 0:128], m_hs[h][:],
                                        ident[:])
                    m8 = small.tile([NCHUNK, 128], F32, tag="m8",
                                    name=f"m8_{h}")
                    nc.vector.tensor_copy(m8[:], trp[0:NCHUNK, 0:128])
                    nc.sync.dma_start(out=qt[sp + 16:sp + 17, :], in_=m8[:])

                # S'^T + exp for both heads, strip-interleaved.
                PTs = {h: pt_pool.tile([128, NCHUNK * N], F16, tag="pt",
                                       name=f"pt{h}")
                       for h in pair}
                for jc in range(NCHUNK):
                    sts = {}
                    for h in pair:
                        sp = 64 * (h % 2)
                        st = psm.tile([128, N], F32, tag="big",
                                      name=f"st_{h}_{jc}")
                        sts[h] = st
                        for half in range(2):
                            sl = slice(half * 512, (half + 1) * 512)
                            nc.tensor.matmul(
                                st[:, sl],
                                kt[sp:sp + 64, jc * 128:(jc + 1) * 128],
                                qt[sp:sp + 64, sl], start=True, stop=True)
                    for h in pair:
                        nc.scalar.activation(
                            PTs[h][:, jc * N:jc * N + N], sts[h][:, :],
                            Exp, bias=0.0, scale=SCALE)

                # PV for both heads (different PSUM col strips).
                for half in range(2):
                    prvs = {}
                    for h in pair:
                        cs = 32 * (h % 3)
                        prv = psr.tile([128, 512], F32, tag="recv",
                                       name=f"prv{h}_{half}")
                        prvs[h] = prv
                        for jc in range(NCHUNK):
                            nc.tensor.matmul(
                                prv[cs:cs + 17, :],
                                V_sb[:, jc, 17 * h:17 * h + 17],
                                PTs[h][:, jc * N + half * 512:
                                        jc * N + (half + 1) * 512],
                                start=(jc == 0), stop=(jc == NCHUNK - 1))
                    for h in pair:
                        cs = 32 * (h % 3)
                        nc.vector.tensor_copy(
                            raw[h // 3][cs:cs + 17,
                                        half * 512:(half + 1) * 512],
                            prvs[h][cs:cs + 17, :])

        # ---- Tail: normalize + output projection. ----
        with tc.tile_pool(name="pst", bufs=2, space="PSUM") as pst, \
                tc.tile_pool(name="pstb", bufs=2, space="PSUM") as pstb:
            for h in range(H):
                rv, cs = raw[h // 3], 32 * (h % 3)
                nc.sync.dma_start(out=recvT[16 * h:16 * h + 16, :],
                                  in_=rv[cs:cs + 16, :])
                nc.sync.dma_start(out=den_sb[h:h + 1, :],
                                  in_=rv[cs + 16:cs + 17, :])
            nc.vector.reciprocal(rden[:], den_sb[:])
            pe_ = pst.tile([128, N], F32, tag="expand")
            for half in range(2):
                sl = slice(half * 512, (half + 1) * 512)
                nc.tensor.matmul(pe_[:, sl], e8[:], rden[:, sl],
                                 start=True, stop=True)
            nc.vector.tensor_mul(recvN[:], recvT[:], pe_[:, :])
            for c in range(NCHUNK):
                po = pstb.tile([128, 128], F32, tag="mha")
                nc.tensor.matmul(po[:, :], recvN[:, c * 128:(c + 1) * 128],
                                 w_sb["wo"][:], start=True, stop=True)
                nc.scalar.copy(mha_sb[:, c, :], po[:, :])
                nc.sync.dma_start(out=d_out[c * 128:(c + 1) * 128, :],
                                  in_=mha_sb[:, c, :])

    nc.finalize()
    return nc


def _permute_weights(Wq, Wk, Wv, Wo):
    """Numpy-side weight layout prep: strip-pack with K=64 zero padding."""
    def strip_pack(W, heads):
        out = np.zeros((X, 128), dtype=np.float32)
        for t, h in enumerate(heads):
            out[:, 64 * t:64 * t + 16] = W[:, h, :]
        return out

    e8c = np.zeros((H, 128), dtype=np.float32)
    for h in range(H):
        e8c[h, 16 * h:16 * h + 16] = 1.0
    d = dict(
        wv=np.ascontiguousarray(Wv.reshape(X, 128)),
        wo=np.ascontiguousarray(Wo.reshape(128, X)),
        e8c=e8c, ones=np.ones((1, N), dtype=np.float32),
    )
    for r in range(NR):
        d[f"wq{r}"] = strip_pack(Wq, [2 * r, 2 * r + 1])
        d[f"wk{r}"] = strip_pack(Wk, [2 * r, 2 * r + 1])
    return d


def kernel(Wq, Wk, Wv, Wo, vec, trace=False):
    global _CACHED_NC
    if _CACHED_NC is None:
        _CACHED_NC = build_nc()
    nc = _CACHED_NC

    w = _permute_weights(np.asarray(Wq, np.float32), np.asarray(Wk, np.float32),
                         np.asarray(Wv, np.float32), np.asarray(Wo, np.float32))
    vec = np.asarray(vec, np.float32)
    in_maps = [dict(w, vec=np.ascontiguousarray(vec[b])) for b in range(B)]
    res = run_bass_kernel_spmd(nc, in_maps, core_ids=list(range(B)),
                               trace=trace)
    out = np.stack([res.results[b]["out"] for b in range(B)])
    if trace:
        return out, res
    return out
